# revision 58
# baseline (speedup 1.0000x reference)
"""Differential cross-attention Trainium2 kernel (8 NeuronCores), v2.

Sharding: 8 cores = (batch b = c//2) x (head-pair group g = c%2).
Core (b, g) computes heads {2g, 2g+4, 2g+1, 2g+5} (two diff-pairs) for
all 1024 queries x 1024 keys of batch b, plus a partial output
projection over its 256 x-columns; the host sums the two partials per
batch.

All matmuls run in bf16 (1 cycle/row on the PE vs 4 for fp32).
Per (pair, q-half) loop over k-tiles m: scores S_T[k,q] per head (K=64
contraction) into PSUM, then the raw RPE bias (streamed bf16 from HBM)
is accumulated on top via an identity matmul (S += I^T T), so a single
ACT exp(S+T) PSUM->SBUF bf16 produces U with no vector-engine multiply.
PV matmuls use 128-wide q stationary tiles against a shared rhs
[V1|V2|1] (129 cols). PSUM: 2x [128,1024] score tiles (8KB) + one
[128,4,512] PV accumulator block (8KB) = 16KB exactly. Combine folds
(1+alpha)/S1 and alpha*lam/S2 into per-q scalars; projection biases ride
the ACT Identity activation's per-partition bias operand.
"""
import sys
sys.path.insert(0, "/opt/trn_rl_repo")
import numpy as np

DIM = 512
H = 8
HD = 64
NQ = 1024
NKV = 1024
MAX_DIST = 128
LAMBDA_INIT = 0.8
N_CORES = 8
SCALE = HD ** -0.5

_COMPILED = {}
DEBUG_DUMPS = False
# timing-probe variants (correctness-breaking, for engine attribution):
#   "" = full kernel, "noexp" = skip ACT exp (u = et*et), "nopv" = skip PV
#   matmuls, "nodma" = no E-bias DMA (et memset once)
VARIANT = ""


def _build(reps=1):
    import concourse.bacc as bacc
    import concourse.mybir as mybir
    from concourse.tile import TileContext
    from concourse.masks import make_identity

    f32 = mybir.dt.float32
    bf16 = mybir.dt.bfloat16
    nc = bacc.Bacc("TRN2", target_bir_lowering=False, debug=False,
                   num_devices=N_CORES)

    xq_T = nc.dram_tensor("xq_T", [DIM, NQ], bf16, kind="ExternalInput")
    xkv_T = nc.dram_tensor("xkv_T", [DIM, NKV], bf16, kind="ExternalInput")
    wq = nc.dram_tensor("wq", [DIM, 256], bf16, kind="ExternalInput")
    wk = nc.dram_tensor("wk", [DIM, 256], bf16, kind="ExternalInput")
    wv = nc.dram_tensor("wv", [DIM, 256], bf16, kind="ExternalInput")
    wp = nc.dram_tensor("wp", [256, DIM], bf16, kind="ExternalInput")
    bq = nc.dram_tensor("bq", [128, 2], f32, kind="ExternalInput")
    bk = nc.dram_tensor("bk", [128, 2], f32, kind="ExternalInput")
    bv = nc.dram_tensor("bv", [128, 256], f32, kind="ExternalInput")
    bp = nc.dram_tensor("bp", [128, 4], f32, kind="ExternalInput")
    alpha = nc.dram_tensor("alpha", [128, 8], f32, kind="ExternalInput")
    lam_in = nc.dram_tensor("lam", [128, 2], f32, kind="ExternalInput")
    fp8 = mybir.dt.float8e4
    # [hp, qh, p, m, j, n]: per-partition lines are 8KB contiguous, so each
    # (hp, qh) quarter loads as one max-bandwidth 1MB DMA
    biasE = nc.dram_tensor("biasE", [2, 2, 128, 8, 2, 512], fp8,
                           kind="ExternalInput")
    out_T = nc.dram_tensor("out_T", [DIM, NQ], bf16, kind="ExternalOutput")
    if DEBUG_DUMPS:
        dbg_q = nc.dram_tensor("dbg_q", [128, 2, NQ], bf16, kind="ExternalOutput")
        dbg_k = nc.dram_tensor("dbg_k", [128, 2, NKV], bf16, kind="ExternalOutput")
        dbg_v = nc.dram_tensor("dbg_v", [128, 8, 256], bf16, kind="ExternalOutput")
        dbg_xc = nc.dram_tensor("dbg_xc", [128, 8, 256], bf16, kind="ExternalOutput")
        dbg_xt = nc.dram_tensor("dbg_xt", [128, 2, NQ], bf16, kind="ExternalOutput")
        dbg_id = nc.dram_tensor("dbg_id", [128, 128], bf16, kind="ExternalOutput")
        dbg_u = nc.dram_tensor("dbg_u", [128, 2, NQ], bf16, kind="ExternalOutput")
        dbg_pv = nc.dram_tensor("dbg_pv", [128, 4, 512], f32, kind="ExternalOutput")

    with TileContext(nc) as tc:
      for _rep in range(reps):
        with (
            tc.tile_pool(name="const", bufs=1) as cpool,
            tc.tile_pool(name="work", bufs=1) as wpool,
            tc.tile_pool(name="stream", bufs=4) as spool,
            tc.tile_pool(name="outp", bufs=2) as opool,
            tc.tile_pool(name="psc", bufs=2, space="PSUM") as scpool,
            tc.tile_pool(name="psacc", bufs=1, space="PSUM") as papool,
        ):
            wq_t = cpool.tile([128, 4, 256], bf16, tag="wq")
            wk_t = cpool.tile([128, 4, 256], bf16, tag="wk")
            wv_t = cpool.tile([128, 4, 256], bf16, tag="wv")
            for w_t, w in ((wk_t, wk), (wv_t, wv), (wq_t, wq)):
                for c in range(4):
                    nc.sync.dma_start(
                        out=w_t[:, c, :],
                        in_=w[:].rearrange("(c p) o -> p c o", p=128)[:, c, :])
            wp_t = cpool.tile([128, 2, DIM], bf16, tag="wp")
            nc.sync.dma_start(
                out=wp_t[:],
                in_=wp[:].rearrange("(c p) o -> p c o", p=128))
            bq_t = cpool.tile([128, 2], f32, tag="bq")
            bk_t = cpool.tile([128, 2], f32, tag="bk")
            bv_t = cpool.tile([128, 256], f32, tag="bv")
            bp_t = cpool.tile([128, 4], f32, tag="bp")
            al_t = cpool.tile([128, 8], f32, tag="al")
            lam_t = cpool.tile([128, 2], f32, tag="lam")
            for t, src in ((bq_t, bq), (bk_t, bk), (bv_t, bv), (bp_t, bp),
                           (al_t, alpha), (lam_t, lam_in)):
                nc.sync.dma_start(out=t[:], in_=src[:])
            al1_t = cpool.tile([128, 8], f32, tag="al1")
            nc.vector.tensor_scalar(out=al1_t[:], in0=al_t[:], scalar1=1.0,
                                    scalar2=None, op0=mybir.AluOpType.add)
            alam_t = cpool.tile([128, 2, 8], f32, tag="alam")
            for hp in range(2):
                nc.vector.tensor_scalar(out=alam_t[:, hp, :], in0=al_t[:],
                                        scalar1=lam_t[:, hp:hp + 1],
                                        scalar2=None,
                                        op0=mybir.AluOpType.mult)
            ident = cpool.tile([128, 128], bf16, tag="ident")
            make_identity(nc, ident[:])
            # identity scaled by 1/64 to undo the host-side x64 fp8 bias range
            identb = cpool.tile([128, 128], bf16, tag="identb")
            nc.vector.tensor_scalar(out=identb[:], in0=ident[:],
                                    scalar1=1.0 / 64.0, scalar2=None,
                                    op0=mybir.AluOpType.mult)
            # dummy exp so the ~2.7us ACT table load overlaps the input DMA
            warm = cpool.tile([128, 1], f32, tag="warm")
            nc.scalar.activation(warm[:], al_t[:, 0:1],
                                 mybir.ActivationFunctionType.Exp)

            xq_t = wpool.tile([128, 4, NQ], bf16, tag="xq")
            xkv_t = wpool.tile([128, 4, NKV], bf16, tag="xkv")
            for c in range(4):
                nc.sync.dma_start(
                    out=xkv_t[:, c, :],
                    in_=xkv_T[:].rearrange("(c p) n -> p c n", p=128)[:, c, :])
            for c in range(4):
                nc.sync.dma_start(
                    out=xq_t[:, c, :],
                    in_=xq_T[:].rearrange("(c p) n -> p c n", p=128)[:, c, :])
            # whole fp8 bias resident in SBUF (32.7KB/partition); 4 blocks
            # of 1MB issued upfront at the max-bandwidth DMA shape
            btile = wpool.tile([128, 2, 2, 8, 2, 512], fp8, tag="bias")
            if VARIANT != "nodma":
                for hp in range(2):
                    for qh in range(2):
                        nc.sync.dma_start(out=btile[:, hp, qh],
                                          in_=biasE[hp, qh])

            # projections, ordered so the hp=0 loop can start as soon as the
            # t=0 K/Q tiles and V/ve exist; t=1 tiles (consumed ~15us later
            # by hp=1) are emitted last.
            q_sb = wpool.tile([128, 2, NQ], bf16, tag="qsb")
            k_sb = wpool.tile([128, 2, NKV], bf16, tag="ksb")
            v_sb = wpool.tile([128, 8, 256], bf16, tag="vsb")

            def _kproj(t):
                ps = scpool.tile([128, NKV], f32, tag="ps", name=f"psk{t}")
                for kh in range(2):
                    for c in range(4):
                        nc.tensor.matmul(
                            ps[:, 512 * kh:512 * (kh + 1)],
                            lhsT=wk_t[:, c, 128 * t:128 * (t + 1)],
                            rhs=xkv_t[:, c, 512 * kh:512 * (kh + 1)],
                            start=(c == 0), stop=(c == 3))
                nc.scalar.activation(k_sb[:, t, :], ps[:],
                                     mybir.ActivationFunctionType.Identity,
                                     bias=bk_t[:, t:t + 1])

            def _qproj(t):
                ps = scpool.tile([128, NQ], f32, tag="ps", name=f"psq{t}")
                for qh in range(2):
                    for c in range(4):
                        nc.tensor.matmul(
                            ps[:, 512 * qh:512 * (qh + 1)],
                            lhsT=wq_t[:, c, 128 * t:128 * (t + 1)],
                            rhs=xq_t[:, c, 512 * qh:512 * (qh + 1)],
                            start=(c == 0), stop=(c == 3))
                nc.scalar.activation(q_sb[:, t, :], ps[:],
                                     mybir.ActivationFunctionType.Identity,
                                     bias=bq_t[:, t:t + 1])

            _kproj(0)
            for m in range(8):
                ps = scpool.tile([128, NQ], f32, tag="ps", name=f"psv{m}")
                for c in range(4):
                    nc.tensor.matmul(ps[:, 0:256],
                                     lhsT=xkv_t[:, c, 128 * m:128 * (m + 1)],
                                     rhs=wv_t[:, c, :],
                                     start=(c == 0), stop=(c == 3))
                nc.vector.tensor_tensor(out=v_sb[:, m, :], in0=ps[:, 0:256],
                                        in1=bv_t[:], op=mybir.AluOpType.add)
            _qproj(0)
            _kproj(1)
            _qproj(1)

            # ve[hp, m] = [V1 | V2 | 1]  (129 cols)
            ve = cpool.tile([128, 2, 8, 129], bf16, tag="ve")
            for hp in range(2):
                for m in range(8):
                    nc.vector.tensor_copy(
                        out=ve[:, hp, m, 0:128],
                        in_=v_sb[:, m, 128 * hp:128 * (hp + 1)])
                    nc.vector.memset(ve[:, hp, m, 128:129], 1.0)

            # PV accumulators: one PSUM bank per 128-q tile i of the current
            # 512-query half, holding A = U1^T [V1|V2|1] at cols 0:129 and
            # B = U2^T [...] at 256:385. A's start=True at m==0 zero-marks
            # the full 2KB bank row (ZERO_REGION_SIZE), so the B matmul
            # sharing the bank must NOT restart the group — its first write
            # lands on pending-zero bytes.
            xcat = wpool.tile([128, 8, 256], bf16, tag="xcat")
            for hp in range(2):
              for qh in range(2):
                pvall = papool.tile([128, 4, 512], f32, tag="pv",
                                    name=f"pvt{hp}_{qh}")
                pvt = [pvall[:, i, :] for i in range(4)]
                for m in range(8):
                    # raw RPE bias [T_h1 half | T_h2 half]; folded into the
                    # scores PSUM via an identity-matmul accumulation
                    # (S += (I/64)^T T'), so exp(S+T) needs no multiply.
                    et = btile[:, hp, qh, m]
                    if VARIANT == "nodma" and hp == 0 and qh == 0 and m == 0:
                        nc.vector.memset(btile[:, 0, 0, 0], 0.0)
                    ss = scpool.tile([128, NQ], f32, tag="ps")
                    for j in range(2):
                        nc.tensor.matmul(
                            ss[:, 512 * j:512 * (j + 1)],
                            lhsT=k_sb[64 * j:64 * (j + 1), hp,
                                      128 * m:128 * (m + 1)],
                            rhs=q_sb[64 * j:64 * (j + 1), hp,
                                     512 * qh:512 * (qh + 1)],
                            start=True, stop=False)
                        nc.tensor.matmul(
                            ss[:, 512 * j:512 * (j + 1)],
                            lhsT=identb[:], rhs=et[:, j, :],
                            start=False, stop=True)
                    u = spool.tile([128, NQ], bf16, tag="u")
                    if VARIANT == "noexp":
                        for j in range(2):
                            nc.vector.tensor_copy(out=u[:, 512 * j:512 * (j + 1)],
                                                  in_=et[:, j, :])
                    else:
                        nc.scalar.activation(u[:], ss[:],
                                             mybir.ActivationFunctionType.Exp)
                    if DEBUG_DUMPS and hp == 0 and qh == 0 and m == 0 \
                            and _rep == 0:
                        nc.sync.dma_start(out=dbg_u[:, 0, :], in_=u[:])
                    first, last = (m == 0), (m == 7)
                    if VARIANT == "nopv":
                        if first:
                            for i in range(4):
                                nc.vector.memset(pvt[i][:], 1.0)
                        continue
                    for i in range(4):
                        nc.tensor.matmul(
                            pvt[i][:, 0:129],
                            lhsT=u[:, 128 * i:128 * (i + 1)],
                            rhs=ve[:, hp, m, :],
                            start=first, stop=last)
                        nc.tensor.matmul(
                            pvt[i][:, 256:385],
                            lhsT=u[:, 512 + 128 * i:512 + 128 * (i + 1)],
                            rhs=ve[:, hp, m, :],
                            start=False, stop=last,
                            skip_group_check=True)
                if DEBUG_DUMPS and hp == 0 and qh == 0 and _rep == 0:
                    for i in range(4):
                        stg = spool.tile([128, 512], f32, tag="dbgstg")
                        nc.vector.tensor_copy(out=stg[:], in_=pvt[i][:])
                        nc.sync.dma_start(out=dbg_pv[:, i, :], in_=stg[:])
                rs1 = spool.tile([128, 4], f32, tag="rs1")
                rs2 = spool.tile([128, 4], f32, tag="rs2")
                nc.vector.reciprocal(rs1[:], pvall[:, :, 128:129])
                nc.vector.reciprocal(rs2[:], pvall[:, :, 384:385])
                for i in range(4):
                    qt = 4 * qh + i
                    tmp1 = spool.tile([128, 64], f32, tag="tmp1")
                    nc.vector.tensor_scalar(out=tmp1[:], in0=pvt[i][:, 0:64],
                                            scalar1=rs1[:, i:i + 1],
                                            scalar2=al1_t[:, qt:qt + 1],
                                            op0=mybir.AluOpType.mult,
                                            op1=mybir.AluOpType.mult)
                    tmp2 = spool.tile([128, 64], f32, tag="tmp2")
                    nc.vector.tensor_scalar(out=tmp2[:], in0=pvt[i][:, 256:320],
                                            scalar1=rs2[:, i:i + 1],
                                            scalar2=alam_t[:, hp, qt:qt + 1],
                                            op0=mybir.AluOpType.mult,
                                            op1=mybir.AluOpType.mult)
                    nc.vector.tensor_tensor(
                        out=xcat[:, qt, 128 * hp:128 * hp + 64],
                        in0=tmp1[:], in1=tmp2[:],
                        op=mybir.AluOpType.subtract)
                    nc.vector.tensor_scalar(
                        out=xcat[:, qt, 128 * hp + 64:128 * hp + 128],
                        in0=pvt[i][:, 320:384],
                        scalar1=rs2[:, i:i + 1], scalar2=None,
                        op0=mybir.AluOpType.mult)

            if DEBUG_DUMPS and _rep == 0:
                nc.sync.dma_start(out=dbg_q[:], in_=q_sb[:])
                nc.sync.dma_start(out=dbg_k[:], in_=k_sb[:])
                nc.sync.dma_start(out=dbg_v[:], in_=v_sb[:])
                nc.sync.dma_start(out=dbg_xc[:], in_=xcat[:])
                nc.sync.dma_start(out=dbg_id[:], in_=ident[:])
            xcat_T = wpool.tile([128, 2, NQ], bf16, tag="xcatT")
            for qc in range(8):
                for dt_ in range(2):
                    pst = scpool.tile([128, 128], bf16, tag="ps")
                    nc.tensor.transpose(
                        out=pst[:],
                        in_=xcat[:, qc, 128 * dt_:128 * (dt_ + 1)],
                        identity=ident[:])
                    nc.vector.tensor_copy(
                        out=xcat_T[:, dt_, 128 * qc:128 * (qc + 1)],
                        in_=pst[:])
            if DEBUG_DUMPS and _rep == 0:
                nc.sync.dma_start(out=dbg_xt[:], in_=xcat_T[:])
            for t in range(4):
                po = scpool.tile([128, NQ], f32, tag="ps")
                for qh in range(2):
                    for c in range(2):
                        nc.tensor.matmul(
                            po[:, 512 * qh:512 * (qh + 1)],
                            lhsT=wp_t[:, c, 128 * t:128 * (t + 1)],
                            rhs=xcat_T[:, c, 512 * qh:512 * (qh + 1)],
                            start=(c == 0), stop=(c == 1))
                ot = opool.tile([128, NQ], bf16, tag="otile")
                nc.scalar.activation(ot[:], po[:],
                                     mybir.ActivationFunctionType.Identity,
                                     bias=bp_t[:, t:t + 1])
                nc.sync.dma_start(
                    out=out_T[:].rearrange("(c p) n -> p c n", p=128)[:, t, :],
                    in_=ot[:])
    nc.compile()
    return nc


def _get_kernel(reps=1):
    key = f"k{reps}-{VARIANT}"
    if key not in _COMPILED:
        _COMPILED[key] = _build(reps)
    return _COMPILED[key]


def _to_bf16(a):
    import ml_dtypes
    return np.asarray(a, dtype=ml_dtypes.bfloat16)


def _to_fp8(a):
    import ml_dtypes
    return np.asarray(np.clip(a, -240.0, 240.0), dtype=ml_dtypes.float8_e4m3)


def _prep_inputs(x_q, x_kv, coords_q, coords_k, alpha_map,
                 Wq, bq, Wk, bk, Wv, bv,
                 lambda_q1, lambda_k1, lambda_q2, lambda_k2,
                 rpe_table, Wp, bp):
    x_q = np.asarray(x_q, dtype=np.float32)
    x_kv = np.asarray(x_kv, dtype=np.float32)
    coords_q = np.asarray(coords_q)
    coords_k = np.asarray(coords_k)
    alpha_map = np.asarray(alpha_map, dtype=np.float32)
    rpe = np.asarray(rpe_table, dtype=np.float32)
    B = x_q.shape[0]

    lam1 = np.exp(np.sum(np.asarray(lambda_q1) * np.asarray(lambda_k1), axis=-1))
    lam2 = np.exp(np.sum(np.asarray(lambda_q2) * np.asarray(lambda_k2), axis=-1))
    lam = (lam1 - lam2 + LAMBDA_INIT).astype(np.float32)

    WqT = np.asarray(Wq, dtype=np.float32).T * SCALE
    WkT = np.asarray(Wk, dtype=np.float32).T
    WvT = np.asarray(Wv, dtype=np.float32).T
    WpT = np.asarray(Wp, dtype=np.float32).T
    bq_s = np.asarray(bq, dtype=np.float32) * SCALE
    bk_s = np.asarray(bk, dtype=np.float32)
    bv_s = np.asarray(bv, dtype=np.float32)
    bp_s = np.asarray(bp, dtype=np.float32)

    # per-batch full-head raw RPE bias, sliced per core below
    E_all = []
    for b in range(B):
        rel = coords_q[b][:, None, :] - coords_k[b][None, :, :] + MAX_DIST
        rel = np.clip(rel, 0, 2 * MAX_DIST)
        idx = rel[..., 0] * (2 * MAX_DIST + 1) + rel[..., 1]  # [Nq, Nk]
        E_all.append(rpe[idx])                                # [Nq, Nk, 8]

    in_maps = []
    for c in range(N_CORES):
        b, g = divmod(c, 2)
        heads = [2 * g, 2 * g + 4, 2 * g + 1, 2 * g + 5]  # slot order
        dcols = np.concatenate([np.arange(h * HD, (h + 1) * HD)
                                for h in heads])
        wq_l = np.ascontiguousarray(WqT[:, dcols])
        wk_l = np.ascontiguousarray(WkT[:, dcols])
        wv_l = np.ascontiguousarray(WvT[:, dcols])
        wp_l = np.ascontiguousarray(WpT[dcols, :])
        bq_l = np.ascontiguousarray(bq_s[dcols].reshape(2, 128).T)
        bk_l = np.ascontiguousarray(bk_s[dcols].reshape(2, 128).T)
        bv_l = np.ascontiguousarray(
            np.tile(bv_s[dcols][None, :], (128, 1)))
        bp_l = np.ascontiguousarray((bp_s / 2.0).reshape(4, 128).T)
        al_l = np.ascontiguousarray(alpha_map[b, :, 0].reshape(8, 128).T)
        lam_l = np.ascontiguousarray(
            np.tile(lam[[2 * g, 2 * g + 1]][None, :], (128, 1)))
        # biasE [hp, qh, 128k, m, j, 512q]: raw bias scaled x64 for fp8 e4m3,
        # laid out so each (hp, qh) quarter DMAs with 8KB-contiguous
        # per-partition lines
        Eb = E_all[b]
        bias_l = np.empty((2, 2, 128, 8, 2, 512), dtype=np.float32)
        for hp in range(2):
            for j, h in enumerate((2 * g + hp, 2 * g + 4 + hp)):
                T = (Eb[:, :, h].T * 64.0).reshape(8, 128, 2, 512)
                bias_l[hp, :, :, :, j] = T.transpose(2, 1, 0, 3)
        in_maps.append({
            "xq_T": _to_bf16(x_q[b].T),
            "xkv_T": _to_bf16(x_kv[b].T),
            "wq": _to_bf16(wq_l), "wk": _to_bf16(wk_l),
            "wv": _to_bf16(wv_l), "wp": _to_bf16(wp_l),
            "bq": bq_l, "bk": bk_l, "bv": bv_l, "bp": bp_l,
            "alpha": al_l, "lam": lam_l,
            "biasE": _to_fp8(bias_l),
        })
    return in_maps


def kernel(x_q, x_kv, coords_q, coords_k, alpha_map,
           Wq, bq, Wk, bk, Wv, bv,
           lambda_q1, lambda_k1, lambda_q2, lambda_k2,
           rpe_table, Wp, bp):
    from concourse.bass_utils import run_bass_kernel_spmd

    nc = _get_kernel()
    in_maps = _prep_inputs(x_q, x_kv, coords_q, coords_k, alpha_map,
                           Wq, bq, Wk, bk, Wv, bv,
                           lambda_q1, lambda_k1, lambda_q2, lambda_k2,
                           rpe_table, Wp, bp)
    res = run_bass_kernel_spmd(nc, in_maps, list(range(N_CORES)))
    B = np.asarray(x_q).shape[0]
    out = np.zeros((B, NQ, DIM), dtype=np.float32)
    for b in range(B):
        out[b] = (res.results[2 * b]["out_T"].astype(np.float32) +
                  res.results[2 * b + 1]["out_T"].astype(np.float32)).T
    return out


# revision 62
# speedup vs baseline: 1.9753x; 1.9753x over previous
"""Differential cross-attention Trainium2 kernel (8 NeuronCores), v2.

Sharding: 8 cores = (batch b = c//2) x (head-pair group g = c%2).
Core (b, g) computes heads {2g, 2g+4, 2g+1, 2g+5} (two diff-pairs) for
all 1024 queries x 1024 keys of batch b, plus a partial output
projection over its 256 x-columns; the host sums the two partials per
batch.

All matmuls run in bf16 (1 cycle/row on the PE vs 4 for fp32).
Per (pair, q-half) loop over k-tiles m: scores S_T[k,q] per head (K=64
contraction) into PSUM, then the raw RPE bias (streamed bf16 from HBM)
is accumulated on top via an identity matmul (S += I^T T), so a single
ACT exp(S+T) PSUM->SBUF bf16 produces U with no vector-engine multiply.
PV matmuls use 128-wide q stationary tiles against a shared rhs
[V1|V2|1] (129 cols). PSUM: 2x [128,1024] score tiles (8KB) + one
[128,4,512] PV accumulator block (8KB) = 16KB exactly. Combine folds
(1+alpha)/S1 and alpha*lam/S2 into per-q scalars; projection biases ride
the ACT Identity activation's per-partition bias operand.
"""
import sys
sys.path.insert(0, "/opt/trn_rl_repo")
import numpy as np

DIM = 512
H = 8
HD = 64
NQ = 1024
NKV = 1024
MAX_DIST = 128
LAMBDA_INIT = 0.8
N_CORES = 8
SCALE = HD ** -0.5

_COMPILED = {}
DEBUG_DUMPS = False
# timing-probe variants (correctness-breaking, for engine attribution):
#   "" = full kernel, "noexp" = skip ACT exp (u = et*et), "nopv" = skip PV
#   matmuls, "nodma" = no E-bias DMA (et memset once)
VARIANT = ""


def _build(reps=1):
    import concourse.bacc as bacc
    import concourse.mybir as mybir
    from concourse.tile import TileContext
    from concourse.masks import make_identity

    f32 = mybir.dt.float32
    bf16 = mybir.dt.bfloat16
    nc = bacc.Bacc("TRN2", target_bir_lowering=False, debug=False,
                   num_devices=N_CORES)

    xq_T = nc.dram_tensor("xq_T", [DIM, NQ], bf16, kind="ExternalInput")
    xkv_T = nc.dram_tensor("xkv_T", [DIM, NKV], bf16, kind="ExternalInput")
    wq = nc.dram_tensor("wq", [DIM, 256], bf16, kind="ExternalInput")
    wk = nc.dram_tensor("wk", [DIM, 256], bf16, kind="ExternalInput")
    wv = nc.dram_tensor("wv", [DIM, 256], bf16, kind="ExternalInput")
    wp = nc.dram_tensor("wp", [256, DIM], bf16, kind="ExternalInput")
    bq = nc.dram_tensor("bq", [128, 2], f32, kind="ExternalInput")
    bk = nc.dram_tensor("bk", [128, 2], f32, kind="ExternalInput")
    bv = nc.dram_tensor("bv", [128, 256], f32, kind="ExternalInput")
    bp = nc.dram_tensor("bp", [128, 4], f32, kind="ExternalInput")
    alpha = nc.dram_tensor("alpha", [128, 8], f32, kind="ExternalInput")
    lam_in = nc.dram_tensor("lam", [128, 2], f32, kind="ExternalInput")
    fp8 = mybir.dt.float8e4
    # [hp, qh, p, m, j, n]: per-partition lines are 8KB contiguous, so each
    # (hp, qh) quarter loads as one max-bandwidth 1MB DMA
    biasE = nc.dram_tensor("biasE", [2, 2, 128, 8, 2, 512], fp8,
                           kind="ExternalInput")
    out_T = nc.dram_tensor("out_T", [DIM, NQ], bf16, kind="ExternalOutput")
    if DEBUG_DUMPS:
        dbg_q = nc.dram_tensor("dbg_q", [128, 2, NQ], bf16, kind="ExternalOutput")
        dbg_k = nc.dram_tensor("dbg_k", [128, 2, NKV], bf16, kind="ExternalOutput")
        dbg_v = nc.dram_tensor("dbg_v", [128, 8, 256], bf16, kind="ExternalOutput")
        dbg_xc = nc.dram_tensor("dbg_xc", [128, 8, 256], bf16, kind="ExternalOutput")
        dbg_xt = nc.dram_tensor("dbg_xt", [128, 2, NQ], bf16, kind="ExternalOutput")
        dbg_id = nc.dram_tensor("dbg_id", [128, 128], bf16, kind="ExternalOutput")
        dbg_u = nc.dram_tensor("dbg_u", [128, 2, NQ], bf16, kind="ExternalOutput")
        dbg_pv = nc.dram_tensor("dbg_pv", [128, 4, 512], f32, kind="ExternalOutput")

    with TileContext(nc) as tc:
      for _rep in range(reps):
        with (
            tc.tile_pool(name="const", bufs=1) as cpool,
            tc.tile_pool(name="work", bufs=1) as wpool,
            tc.tile_pool(name="stream", bufs=4) as spool,
            tc.tile_pool(name="outp", bufs=2) as opool,
            tc.tile_pool(name="psc", bufs=2, space="PSUM") as scpool,
            tc.tile_pool(name="psacc", bufs=1, space="PSUM") as papool,
        ):
            wq_t = cpool.tile([128, 4, 256], bf16, tag="wq")
            wk_t = cpool.tile([128, 4, 256], bf16, tag="wk")
            wv_t = cpool.tile([128, 4, 256], bf16, tag="wv")
            for w_t, w in ((wk_t, wk), (wv_t, wv), (wq_t, wq)):
                for c in range(4):
                    nc.sync.dma_start(
                        out=w_t[:, c, :],
                        in_=w[:].rearrange("(c p) o -> p c o", p=128)[:, c, :])
            wp_t = cpool.tile([128, 2, DIM], bf16, tag="wp")
            nc.sync.dma_start(
                out=wp_t[:],
                in_=wp[:].rearrange("(c p) o -> p c o", p=128))
            bq_t = cpool.tile([128, 2], f32, tag="bq")
            bk_t = cpool.tile([128, 2], f32, tag="bk")
            bv_t = cpool.tile([128, 256], f32, tag="bv")
            bp_t = cpool.tile([128, 4], f32, tag="bp")
            al_t = cpool.tile([128, 8], f32, tag="al")
            lam_t = cpool.tile([128, 2], f32, tag="lam")
            for t, src in ((bq_t, bq), (bk_t, bk), (bv_t, bv), (bp_t, bp),
                           (al_t, alpha), (lam_t, lam_in)):
                nc.sync.dma_start(out=t[:], in_=src[:])
            al1_t = cpool.tile([128, 8], f32, tag="al1")
            nc.vector.tensor_scalar(out=al1_t[:], in0=al_t[:], scalar1=1.0,
                                    scalar2=None, op0=mybir.AluOpType.add)
            alam_t = cpool.tile([128, 2, 8], f32, tag="alam")
            for hp in range(2):
                nc.vector.tensor_scalar(out=alam_t[:, hp, :], in0=al_t[:],
                                        scalar1=lam_t[:, hp:hp + 1],
                                        scalar2=None,
                                        op0=mybir.AluOpType.mult)
            ident = cpool.tile([128, 128], bf16, tag="ident")
            make_identity(nc, ident[:])
            # identity scaled by 1/64 to undo the host-side x64 fp8 bias range
            identb = cpool.tile([128, 128], bf16, tag="identb")
            nc.vector.tensor_scalar(out=identb[:], in0=ident[:],
                                    scalar1=1.0 / 64.0, scalar2=None,
                                    op0=mybir.AluOpType.mult)
            # dummy exp so the ~2.7us ACT table load overlaps the input DMA
            warm = cpool.tile([128, 1], f32, tag="warm")
            nc.scalar.activation(warm[:], al_t[:, 0:1],
                                 mybir.ActivationFunctionType.Exp)

            xq_t = wpool.tile([128, 4, NQ], bf16, tag="xq")
            xkv_t = wpool.tile([128, 4, NKV], bf16, tag="xkv")
            for c in range(4):
                nc.sync.dma_start(
                    out=xkv_t[:, c, :],
                    in_=xkv_T[:].rearrange("(c p) n -> p c n", p=128)[:, c, :])
            for c in range(4):
                nc.sync.dma_start(
                    out=xq_t[:, c, :],
                    in_=xq_T[:].rearrange("(c p) n -> p c n", p=128)[:, c, :])
            # whole fp8 bias resident in SBUF (32.7KB/partition); 4 blocks
            # of 1MB issued upfront at the max-bandwidth DMA shape
            btile = wpool.tile([128, 2, 2, 8, 2, 512], fp8, tag="bias")
            if VARIANT != "nodma":
                for hp in range(2):
                    for qh in range(2):
                        nc.sync.dma_start(out=btile[:, hp, qh],
                                          in_=biasE[hp, qh])

            # projections, ordered so the hp=0 loop can start as soon as the
            # t=0 K/Q tiles and V/ve exist; t=1 tiles (consumed ~15us later
            # by hp=1) are emitted last.
            q_sb = wpool.tile([128, 2, NQ], bf16, tag="qsb")
            k_sb = wpool.tile([128, 2, NKV], bf16, tag="ksb")
            v_sb = wpool.tile([128, 8, 256], bf16, tag="vsb")

            def _kproj(t):
                ps = scpool.tile([128, NKV], f32, tag="ps", name=f"psk{t}")
                for kh in range(2):
                    for c in range(4):
                        nc.tensor.matmul(
                            ps[:, 512 * kh:512 * (kh + 1)],
                            lhsT=wk_t[:, c, 128 * t:128 * (t + 1)],
                            rhs=xkv_t[:, c, 512 * kh:512 * (kh + 1)],
                            start=(c == 0), stop=(c == 3))
                nc.scalar.activation(k_sb[:, t, :], ps[:],
                                     mybir.ActivationFunctionType.Identity,
                                     bias=bk_t[:, t:t + 1])

            def _qproj(t):
                ps = scpool.tile([128, NQ], f32, tag="ps", name=f"psq{t}")
                for qh in range(2):
                    for c in range(4):
                        nc.tensor.matmul(
                            ps[:, 512 * qh:512 * (qh + 1)],
                            lhsT=wq_t[:, c, 128 * t:128 * (t + 1)],
                            rhs=xq_t[:, c, 512 * qh:512 * (qh + 1)],
                            start=(c == 0), stop=(c == 3))
                nc.scalar.activation(q_sb[:, t, :], ps[:],
                                     mybir.ActivationFunctionType.Identity,
                                     bias=bq_t[:, t:t + 1])

            _kproj(0)
            for m in range(8):
                ps = scpool.tile([128, NQ], f32, tag="ps", name=f"psv{m}")
                for c in range(4):
                    nc.tensor.matmul(ps[:, 0:256],
                                     lhsT=xkv_t[:, c, 128 * m:128 * (m + 1)],
                                     rhs=wv_t[:, c, :],
                                     start=(c == 0), stop=(c == 3))
                nc.vector.tensor_tensor(out=v_sb[:, m, :], in0=ps[:, 0:256],
                                        in1=bv_t[:], op=mybir.AluOpType.add)
            _qproj(0)
            _kproj(1)
            _qproj(1)

            # ve[hp, m] = [V1 | V2 | 1]  (129 cols)
            ve = cpool.tile([128, 2, 8, 129], bf16, tag="ve")
            for hp in range(2):
                for m in range(8):
                    nc.vector.tensor_copy(
                        out=ve[:, hp, m, 0:128],
                        in_=v_sb[:, m, 128 * hp:128 * (hp + 1)])
                    nc.vector.memset(ve[:, hp, m, 128:129], 1.0)

            # PV accumulators: one PSUM bank per 128-q tile i of the current
            # 512-query half, holding A = U1^T [V1|V2|1] at cols 0:129 and
            # B = U2^T [...] at 256:385. A's start=True at m==0 zero-marks
            # the full 2KB bank row (ZERO_REGION_SIZE), so the B matmul
            # sharing the bank must NOT restart the group — its first write
            # lands on pending-zero bytes.
            xcat = wpool.tile([128, 8, 256], bf16, tag="xcat")
            xcat_T = wpool.tile([128, 2, NQ], bf16, tag="xcatT")
            for hp in range(2):
              for qh in range(2):
                pvall = papool.tile([128, 4, 512], f32, tag="pv",
                                    name=f"pvt{hp}_{qh}")
                pvt = [pvall[:, i, :] for i in range(4)]
                for m in range(8):
                    # raw RPE bias [T_h1 half | T_h2 half]; folded into the
                    # scores PSUM via an identity-matmul accumulation
                    # (S += (I/64)^T T'), so exp(S+T) needs no multiply.
                    et = btile[:, hp, qh, m]
                    if VARIANT == "nodma" and hp == 0 and qh == 0 and m == 0:
                        nc.vector.memset(btile[:, 0, 0, 0], 0.0)
                    ss = scpool.tile([128, NQ], f32, tag="ps")
                    for j in range(2):
                        nc.tensor.matmul(
                            ss[:, 512 * j:512 * (j + 1)],
                            lhsT=k_sb[64 * j:64 * (j + 1), hp,
                                      128 * m:128 * (m + 1)],
                            rhs=q_sb[64 * j:64 * (j + 1), hp,
                                     512 * qh:512 * (qh + 1)],
                            start=True, stop=False)
                        nc.tensor.matmul(
                            ss[:, 512 * j:512 * (j + 1)],
                            lhsT=identb[:], rhs=et[:, j, :],
                            start=False, stop=True)
                    u = spool.tile([128, NQ], bf16, tag="u")
                    if VARIANT == "noexp":
                        for j in range(2):
                            nc.vector.tensor_copy(out=u[:, 512 * j:512 * (j + 1)],
                                                  in_=et[:, j, :])
                    else:
                        nc.scalar.activation(u[:], ss[:],
                                             mybir.ActivationFunctionType.Exp)
                    if DEBUG_DUMPS and hp == 0 and qh == 0 and m == 0 \
                            and _rep == 0:
                        nc.sync.dma_start(out=dbg_u[:, 0, :], in_=u[:])
                    first, last = (m == 0), (m == 7)
                    if VARIANT == "nopv":
                        if first:
                            for i in range(4):
                                nc.vector.memset(pvt[i][:], 1.0)
                        continue
                    for i in range(4):
                        nc.tensor.matmul(
                            pvt[i][:, 0:129],
                            lhsT=u[:, 128 * i:128 * (i + 1)],
                            rhs=ve[:, hp, m, :],
                            start=first, stop=last)
                        nc.tensor.matmul(
                            pvt[i][:, 256:385],
                            lhsT=u[:, 512 + 128 * i:512 + 128 * (i + 1)],
                            rhs=ve[:, hp, m, :],
                            start=False, stop=last,
                            skip_group_check=True)
                if DEBUG_DUMPS and hp == 0 and qh == 0 and _rep == 0:
                    for i in range(4):
                        stg = spool.tile([128, 512], f32, tag="dbgstg")
                        nc.vector.tensor_copy(out=stg[:], in_=pvt[i][:])
                        nc.sync.dma_start(out=dbg_pv[:, i, :], in_=stg[:])
                rs1 = spool.tile([128, 4], f32, tag="rs1")
                rs2 = spool.tile([128, 4], f32, tag="rs2")
                nc.vector.reciprocal(rs1[:], pvall[:, :, 128:129])
                nc.vector.reciprocal(rs2[:], pvall[:, :, 384:385])
                for i in range(4):
                    qt = 4 * qh + i
                    tmp1 = spool.tile([128, 64], f32, tag="tmp1")
                    nc.vector.tensor_scalar(out=tmp1[:], in0=pvt[i][:, 0:64],
                                            scalar1=rs1[:, i:i + 1],
                                            scalar2=al1_t[:, qt:qt + 1],
                                            op0=mybir.AluOpType.mult,
                                            op1=mybir.AluOpType.mult)
                    tmp2 = spool.tile([128, 64], f32, tag="tmp2")
                    nc.vector.tensor_scalar(out=tmp2[:], in0=pvt[i][:, 256:320],
                                            scalar1=rs2[:, i:i + 1],
                                            scalar2=alam_t[:, hp, qt:qt + 1],
                                            op0=mybir.AluOpType.mult,
                                            op1=mybir.AluOpType.mult)
                    nc.vector.tensor_tensor(
                        out=xcat[:, qt, 128 * hp:128 * hp + 64],
                        in0=tmp1[:], in1=tmp2[:],
                        op=mybir.AluOpType.subtract)
                    nc.vector.tensor_scalar(
                        out=xcat[:, qt, 128 * hp + 64:128 * hp + 128],
                        in0=pvt[i][:, 320:384],
                        scalar1=rs2[:, i:i + 1], scalar2=None,
                        op0=mybir.AluOpType.mult)
                # transpose hp=0's completed xcat block mid-way through the
                # hp=1 stream: its combine finished a full phase ago, so
                # these insert no PE stall and come off the tail
                if hp == 1 and qh == 0:
                    for qc in range(8):
                        pst = scpool.tile([128, 128], bf16, tag="ps")
                        nc.tensor.transpose(
                            out=pst[:],
                            in_=xcat[:, qc, 0:128],
                            identity=ident[:])
                        nc.vector.tensor_copy(
                            out=xcat_T[:, 0, 128 * qc:128 * (qc + 1)],
                            in_=pst[:])

            for qc in range(8):
                pst = scpool.tile([128, 128], bf16, tag="ps")
                nc.tensor.transpose(
                    out=pst[:],
                    in_=xcat[:, qc, 128:256],
                    identity=ident[:])
                nc.vector.tensor_copy(
                    out=xcat_T[:, 1, 128 * qc:128 * (qc + 1)],
                    in_=pst[:])
            if DEBUG_DUMPS and _rep == 0:
                nc.sync.dma_start(out=dbg_q[:], in_=q_sb[:])
                nc.sync.dma_start(out=dbg_k[:], in_=k_sb[:])
                nc.sync.dma_start(out=dbg_v[:], in_=v_sb[:])
                nc.sync.dma_start(out=dbg_xc[:], in_=xcat[:])
                nc.sync.dma_start(out=dbg_id[:], in_=ident[:])
            if DEBUG_DUMPS and _rep == 0:
                nc.sync.dma_start(out=dbg_xt[:], in_=xcat_T[:])
            for t in range(4):
                po = scpool.tile([128, NQ], f32, tag="ps")
                for qh in range(2):
                    for c in range(2):
                        nc.tensor.matmul(
                            po[:, 512 * qh:512 * (qh + 1)],
                            lhsT=wp_t[:, c, 128 * t:128 * (t + 1)],
                            rhs=xcat_T[:, c, 512 * qh:512 * (qh + 1)],
                            start=(c == 0), stop=(c == 1))
                ot = opool.tile([128, NQ], bf16, tag="otile")
                nc.scalar.activation(ot[:], po[:],
                                     mybir.ActivationFunctionType.Identity,
                                     bias=bp_t[:, t:t + 1])
                nc.sync.dma_start(
                    out=out_T[:].rearrange("(c p) n -> p c n", p=128)[:, t, :],
                    in_=ot[:])
    nc.compile()
    return nc


def _get_kernel(reps=1):
    key = f"k{reps}-{VARIANT}"
    if key not in _COMPILED:
        _COMPILED[key] = _build(reps)
    return _COMPILED[key]


def _to_bf16(a):
    import ml_dtypes
    return np.asarray(a, dtype=ml_dtypes.bfloat16)


def _to_fp8(a):
    import ml_dtypes
    return np.asarray(np.clip(a, -240.0, 240.0), dtype=ml_dtypes.float8_e4m3)


def _prep_inputs(x_q, x_kv, coords_q, coords_k, alpha_map,
                 Wq, bq, Wk, bk, Wv, bv,
                 lambda_q1, lambda_k1, lambda_q2, lambda_k2,
                 rpe_table, Wp, bp):
    x_q = np.asarray(x_q, dtype=np.float32)
    x_kv = np.asarray(x_kv, dtype=np.float32)
    coords_q = np.asarray(coords_q)
    coords_k = np.asarray(coords_k)
    alpha_map = np.asarray(alpha_map, dtype=np.float32)
    rpe = np.asarray(rpe_table, dtype=np.float32)
    B = x_q.shape[0]

    lam1 = np.exp(np.sum(np.asarray(lambda_q1) * np.asarray(lambda_k1), axis=-1))
    lam2 = np.exp(np.sum(np.asarray(lambda_q2) * np.asarray(lambda_k2), axis=-1))
    lam = (lam1 - lam2 + LAMBDA_INIT).astype(np.float32)

    WqT = np.asarray(Wq, dtype=np.float32).T * SCALE
    WkT = np.asarray(Wk, dtype=np.float32).T
    WvT = np.asarray(Wv, dtype=np.float32).T
    WpT = np.asarray(Wp, dtype=np.float32).T
    bq_s = np.asarray(bq, dtype=np.float32) * SCALE
    bk_s = np.asarray(bk, dtype=np.float32)
    bv_s = np.asarray(bv, dtype=np.float32)
    bp_s = np.asarray(bp, dtype=np.float32)

    # per-batch full-head raw RPE bias, sliced per core below
    E_all = []
    for b in range(B):
        rel = coords_q[b][:, None, :] - coords_k[b][None, :, :] + MAX_DIST
        rel = np.clip(rel, 0, 2 * MAX_DIST)
        idx = rel[..., 0] * (2 * MAX_DIST + 1) + rel[..., 1]  # [Nq, Nk]
        E_all.append(rpe[idx])                                # [Nq, Nk, 8]

    in_maps = []
    for c in range(N_CORES):
        b, g = divmod(c, 2)
        heads = [2 * g, 2 * g + 4, 2 * g + 1, 2 * g + 5]  # slot order
        dcols = np.concatenate([np.arange(h * HD, (h + 1) * HD)
                                for h in heads])
        wq_l = np.ascontiguousarray(WqT[:, dcols])
        wk_l = np.ascontiguousarray(WkT[:, dcols])
        wv_l = np.ascontiguousarray(WvT[:, dcols])
        wp_l = np.ascontiguousarray(WpT[dcols, :])
        bq_l = np.ascontiguousarray(bq_s[dcols].reshape(2, 128).T)
        bk_l = np.ascontiguousarray(bk_s[dcols].reshape(2, 128).T)
        bv_l = np.ascontiguousarray(
            np.tile(bv_s[dcols][None, :], (128, 1)))
        bp_l = np.ascontiguousarray((bp_s / 2.0).reshape(4, 128).T)
        al_l = np.ascontiguousarray(alpha_map[b, :, 0].reshape(8, 128).T)
        lam_l = np.ascontiguousarray(
            np.tile(lam[[2 * g, 2 * g + 1]][None, :], (128, 1)))
        # biasE [hp, qh, 128k, m, j, 512q]: raw bias scaled x64 for fp8 e4m3,
        # laid out so each (hp, qh) quarter DMAs with 8KB-contiguous
        # per-partition lines
        Eb = E_all[b]
        bias_l = np.empty((2, 2, 128, 8, 2, 512), dtype=np.float32)
        for hp in range(2):
            for j, h in enumerate((2 * g + hp, 2 * g + 4 + hp)):
                T = (Eb[:, :, h].T * 64.0).reshape(8, 128, 2, 512)
                bias_l[hp, :, :, :, j] = T.transpose(2, 1, 0, 3)
        in_maps.append({
            "xq_T": _to_bf16(x_q[b].T),
            "xkv_T": _to_bf16(x_kv[b].T),
            "wq": _to_bf16(wq_l), "wk": _to_bf16(wk_l),
            "wv": _to_bf16(wv_l), "wp": _to_bf16(wp_l),
            "bq": bq_l, "bk": bk_l, "bv": bv_l, "bp": bp_l,
            "alpha": al_l, "lam": lam_l,
            "biasE": _to_fp8(bias_l),
        })
    return in_maps


def kernel(x_q, x_kv, coords_q, coords_k, alpha_map,
           Wq, bq, Wk, bk, Wv, bv,
           lambda_q1, lambda_k1, lambda_q2, lambda_k2,
           rpe_table, Wp, bp):
    from concourse.bass_utils import run_bass_kernel_spmd

    nc = _get_kernel()
    in_maps = _prep_inputs(x_q, x_kv, coords_q, coords_k, alpha_map,
                           Wq, bq, Wk, bk, Wv, bv,
                           lambda_q1, lambda_k1, lambda_q2, lambda_k2,
                           rpe_table, Wp, bp)
    res = run_bass_kernel_spmd(nc, in_maps, list(range(N_CORES)))
    B = np.asarray(x_q).shape[0]
    out = np.zeros((B, NQ, DIM), dtype=np.float32)
    for b in range(B):
        out[b] = (res.results[2 * b]["out_T"].astype(np.float32) +
                  res.results[2 * b + 1]["out_T"].astype(np.float32)).T
    return out
